# revision 1
# baseline (speedup 1.0000x reference)
"""AttentionalFactorizationMachine kernel for 8 Trainium2 NeuronCores.

Data-parallel: batch dim (1024) sharded 128/core across 8 cores; the small
128x128 attention weight + bias are replicated. The per-core program is the
fused AFM pipeline (pairwise products -> attention MLP + relu -> scores ->
softmax over pairs -> weighted pairwise sum), compiled for the NeuronCores.
"""

import numpy as np
import jax
import jax.numpy as jnp
from jax.sharding import Mesh, PartitionSpec, NamedSharding

B, F, D, A = 1024, 33, 128, 128
N_CORES = 8
_ROW, _COL = np.triu_indices(F, k=1)  # 528 pairs, row-major contiguous by row


def _afm(gnn, x, W, b):
    # gnn: [Bc, A], x: [Bc, F, D], W: [A, D], b: [A]
    inner = x[:, _ROW, :] * x[:, _COL, :]                      # [Bc, P, D]
    fm = jax.nn.relu(jnp.einsum("bpd,ad->bpa", inner, W) + b)  # [Bc, P, A]
    scores = jnp.einsum("ba,bpa->bp", gnn, fm)                 # [Bc, P]
    attn = jax.nn.softmax(scores, axis=1)                      # [Bc, P]
    out = jnp.einsum("bp,bpd->bd", attn, inner) * 100.0        # [Bc, D]
    return jnp.concatenate([gnn, out], axis=1)                 # [Bc, A+D]


_COMPILED = None


def _get_compiled():
    global _COMPILED
    if _COMPILED is None:
        devs = jax.devices()[:N_CORES]
        mesh = Mesh(np.asarray(devs), ("core",))
        shard = NamedSharding(mesh, PartitionSpec("core"))
        repl = NamedSharding(mesh, PartitionSpec())
        _COMPILED = jax.jit(
            _afm,
            in_shardings=(shard, shard, repl, repl),
            out_shardings=shard,
        )
    return _COMPILED


def kernel(gnn_feature, x, attn_W, attn_b):
    f = _get_compiled()
    out = f(
        jnp.asarray(gnn_feature, dtype=jnp.float32),
        jnp.asarray(x, dtype=jnp.float32),
        jnp.asarray(attn_W, dtype=jnp.float32),
        jnp.asarray(attn_b, dtype=jnp.float32),
    )
    return np.asarray(jax.device_get(out)).astype(np.float32)
